# revision 14
# baseline (speedup 1.0000x reference)
"""2x nearest-neighbor upsample of complex (real+imag) NHWC images on 8 trn2 cores.

out[t, b, i, j, c] = x_t[b, i // 2, j // 2, c]   (t = real/imag)

Strategy (data-parallel over batch, 2 images per core):
  - load half an input image row-block into SBUF ([128 rows, 64*64 f32] per half)
  - expand W in SBUF with DVE broadcast copies (each 64-float C-chunk doubled),
    materializing BOTH duplicated output rows in one [128, 16384] tile
  - one store per tile: 3-dim DRAM AP [row i: 128][copy r: 2][8192 contig]
    (walrus caps sync waits per instruction, so fewer DMAs per tile = fewer
    distinct DMA-lane semaphores any instruction must wait on)
HBM traffic per core = 16 MiB read + 64 MiB write (the minimum).
"""

import sys

import numpy as np

if "/opt/trn_rl_repo" not in sys.path:
    sys.path.insert(0, "/opt/trn_rl_repo")

import concourse.bass as bass
import concourse.mybir as mybir
from concourse.bass_utils import run_bass_kernel_spmd
from concourse.tile import TileContext
from concourse.tile_rust import add_dep_helper

F32 = mybir.dt.float32

B, H, W, C = 16, 128, 128, 64
N_CORES = 8
BPC = B // N_CORES  # images per core
N_ITERS = 2 * BPC * 2  # (real+imag) x images x W-halves


def _build() -> bass.Bass:
    nc = bass.Bass("TRN2", debug=False)
    xr = nc.dram_tensor("x_real", [BPC, H, W, C], F32, kind="ExternalInput").ap()
    xi = nc.dram_tensor("x_imag", [BPC, H, W, C], F32, kind="ExternalInput").ap()
    out = nc.dram_tensor(
        "out", [2, BPC, 2 * H, 2 * W, C], F32, kind="ExternalOutput"
    ).ap()
    WH = W // 2  # input W columns per half-tile

    HALF = 2 * WH * C  # expanded half-row length (8192 f32 = 32 KB)

    # walrus codegen allows exactly ONE sync-wait command per engine
    # instruction (multi-wait is only legal on Drain/EventSemaphore).
    # Tile emits a wait only when the issuing engine has not already
    # observed that semaphore tick through an earlier *real* instruction's
    # wait (NoOps don't count). So every instruction below is budgeted to
    # observe at most one fresh tick:
    #   DVE, per iter k: vabs1 observes store_{k-2}'s DMA lane (tout slot
    #   WAR), vabs2 observes ACT's sem (probe-read WAR), cp0 observes the
    #   load lane (RAW), cp1 re-observes DVE's own sem (watermark: makes
    #   all older same-engine WAW ticks "observed").
    #   ACT, per iter k: aabs (2-element probe copy) observes DVE's sem at
    #   cp1_k's tick, so the store and later loads need no DVE wait; the
    #   store observes its DMA lane; the load observes its slot's WAW lane.
    # vabs1/vabs2 write pad cells past the stored region; aabs probes two
    # cells of cp1's region and writes per-iter distinct dummy columns so
    # none of the absorbers create same-cell WAW chains of their own.
    with TileContext(nc) as tc:
        with (
            tc.tile_pool(name="pin", bufs=2) as pin,
            tc.tile_pool(name="pout", bufs=2) as pout,
            tc.tile_pool(name="pdummy", bufs=1) as pdummy,
        ):
            dummy = pdummy.tile([H, 2 * N_ITERS], F32, name="dummy")
            vdummy = pdummy.tile([H, 2 * N_ITERS], F32, name="vdummy")
            spdummy = pdummy.tile([1, 16], F32, name="spdummy")
            stores = []
            aabses = []
            dmas = []
            cps_all = []
            k = 0
            for t, x in enumerate((xr, xi)):
                for b in range(BPC):
                    # partition i holds input row i, feeding output rows 2i, 2i+1
                    ov = out[t, b].rearrange("(i r) w c -> i r (w c)", r=2)
                    for h in range(2):
                        tin = pin.tile([H, WH * C], F32, name="tin")
                        ld = nc.scalar.dma_start(
                            out=tin, in_=x[b, :, h * WH : (h + 1) * WH, :]
                        )
                        if aabses:
                            add_dep_helper(
                                ld.ins, aabses[-1].ins, sync=False,
                                reason="load issues after ACT observed DVE",
                            )
                        tout = pout.tile([H, 2 * HALF], F32, name="tout")
                        # per-iter distinct scratch cells -> the absorbers
                        # have no WAW chains of their own, and they don't
                        # touch tout (the slot-release bundle must land on
                        # cp0, after both absorbers already observed it)
                        vabs1 = nc.vector.memset(vdummy[:1, 2 * k : 2 * k + 1], 0.0)
                        vabs2 = nc.vector.memset(
                            vdummy[:1, 2 * k + 1 : 2 * k + 2], 0.0
                        )
                        if k >= 2:
                            add_dep_helper(
                                vabs1.ins, stores[k - 2].ins, sync=True,
                                reason="absorb tout slot WAR (store lane)",
                            )
                        if k >= 1:
                            add_dep_helper(
                                vabs2.ins, aabses[k - 1].ins, sync=True,
                                reason="absorb probe WAR (ACT sem)",
                            )
                        src = (
                            tin.rearrange("p (w c) -> p w c", c=C)
                            .unsqueeze(2)
                            .broadcast_to([H, WH, 2, C])
                        )
                        cps = []
                        for r in range(2):
                            dst = tout[:, r * HALF : (r + 1) * HALF].rearrange(
                                "p (w s c) -> p w s c", s=2, c=C
                            )
                            cp = nc.vector.tensor_copy(out=dst, in_=src)
                            add_dep_helper(
                                cp.ins, vabs1.ins, sync=False,
                                reason="absorbers run before copies",
                            )
                            add_dep_helper(
                                cp.ins, vabs2.ins, sync=False,
                                reason="absorbers run before copies",
                            )
                            cps.append(cp)
                        add_dep_helper(
                            cps[1].ins, cps[0].ins, sync=True,
                            reason="DVE self-sem watermark",
                        )
                        # 2-element probe inside cp1's region; RAW on cp1's
                        # tick (cumulative sem also covers cp0)
                        aabs = nc.scalar.copy(
                            out=dummy[:1, 2 * k : 2 * k + 2],
                            in_=tout[:1, HALF + 1 : HALF + 3],
                        )
                        st = nc.scalar.dma_start(
                            out=ov[:, :, h * HALF : (h + 1) * HALF], in_=tout
                        )
                        add_dep_helper(
                            st.ins, aabs.ins, sync=False,
                            reason="probe runs before store",
                        )
                        stores.append(st)
                        aabses.append(aabs)
                        dmas.append(ld)
                        dmas.append(st)
                        cps_all.extend(cps)
                        k += 1
            # Kernel-tail absorbers: Tile's final SP drain waits on every
            # outstanding proc (DVE + ACT + 8 DMA lanes = 10 waits), but SP
            # executed nothing, so its drain lowers to a 1-wait NOP struct.
            # Pre-observe each proc with one 4-byte SP write per tick.
            tail_deps = dmas[-8:] + [aabses[-1], cps_all[-1]]
            for j, dep in enumerate(tail_deps):
                wr = nc.sync.write(spdummy[:1, j : j + 1], b"\x00\x00\x00\x00")
                add_dep_helper(
                    wr.ins, dep.ins, sync=True,
                    reason="pre-observe outstanding procs for tail drain",
                )
    return nc


_NC_CACHE: bass.Bass | None = None


def _get_nc() -> bass.Bass:
    global _NC_CACHE
    if _NC_CACHE is None:
        _NC_CACHE = _build()
    return _NC_CACHE


def _run(x_real: np.ndarray, x_imag: np.ndarray, **spmd_kwargs):
    x_real = np.ascontiguousarray(np.asarray(x_real, dtype=np.float32))
    x_imag = np.ascontiguousarray(np.asarray(x_imag, dtype=np.float32))
    assert x_real.shape == (B, H, W, C), x_real.shape
    assert x_imag.shape == (B, H, W, C), x_imag.shape
    in_maps = [
        {
            "x_real": x_real[c * BPC : (c + 1) * BPC],
            "x_imag": x_imag[c * BPC : (c + 1) * BPC],
        }
        for c in range(N_CORES)
    ]
    res = run_bass_kernel_spmd(
        _get_nc(), in_maps, core_ids=list(range(N_CORES)), **spmd_kwargs
    )
    full = np.concatenate([r["out"] for r in res.results], axis=1)
    return full, res


def kernel(x_real: np.ndarray, x_imag: np.ndarray) -> np.ndarray:
    full, _ = _run(x_real, x_imag)
    return full


# revision 23
# speedup vs baseline: 1.2813x; 1.2813x over previous
"""2x nearest-neighbor upsample of complex (real+imag) NHWC images on 8 trn2 cores.

out[t, b, i, j, c] = x_t[b, i // 2, j // 2, c]   (t = real/imag)

Strategy (data-parallel over batch, 2 images per core):
  - load half an input image row-block into SBUF ([128 rows, 64*64 f32] per half)
  - expand W in SBUF with DVE broadcast copies (each 64-float C-chunk doubled),
    materializing BOTH duplicated output rows in one [128, 16384] tile
  - one store per tile: 3-dim DRAM AP [row i: 128][copy r: 2][8192 contig]
    (walrus caps sync waits per instruction, so fewer DMAs per tile = fewer
    distinct DMA-lane semaphores any instruction must wait on)
HBM traffic per core = 16 MiB read + 64 MiB write (the minimum).
"""

import sys

import numpy as np

if "/opt/trn_rl_repo" not in sys.path:
    sys.path.insert(0, "/opt/trn_rl_repo")

import concourse.bass as bass
import concourse.bass_isa as bass_isa
import concourse.mybir as mybir
import concourse.tile_sem_assignment as _tsa
from concourse.bass_utils import run_bass_kernel_spmd
from concourse.tile import TileContext
from concourse.tile_rust import add_dep_helper

# Partition HWDGE DMA-completion semaphore lanes by issuing engine: SP
# (loads) on lane 0, ACT (stores) on lanes 2-7. Each lane then carries
# DMAs from a single HWDGE FIFO ring (per-lane completion order is
# trivially sound), and a DMA's own-lane predecessor is always one the
# issuing engine has already observed — keeping every DMA at the 1
# sync-wait walrus codegen allows.
_orig_assign_tick = _tsa.TileClockTick._assign_tick


def _assign_tick_lane_split(self, inst):
    if isinstance(inst, _tsa.DMAInst) and not isinstance(
        inst, bass_isa.UserSyncedRemoteDMADescs
    ):
        if inst.engine == mybir.EngineType.Pool:
            self.next_sw_dma_idx = 0
        elif inst.engine == mybir.EngineType.Activation:
            r = getattr(self, "_act_lane_rr", 0)
            self.next_hw_dma_idx = 2 + r
            self._act_lane_rr = (r + 1) % 6
    return _orig_assign_tick(self, inst)


_tsa.TileClockTick._assign_tick = _assign_tick_lane_split

F32 = mybir.dt.float32

B, H, W, C = 16, 128, 128, 64
N_CORES = 8
BPC = B // N_CORES  # images per core
N_ITERS = 2 * BPC * 2  # (real+imag) x images x W-halves


def _build() -> bass.Bass:
    nc = bass.Bass("TRN2", debug=False)
    xr = nc.dram_tensor("x_real", [BPC, H, W, C], F32, kind="ExternalInput").ap()
    xi = nc.dram_tensor("x_imag", [BPC, H, W, C], F32, kind="ExternalInput").ap()
    out = nc.dram_tensor(
        "out", [2, BPC, 2 * H, 2 * W, C], F32, kind="ExternalOutput"
    ).ap()
    WH = W // 2  # input W columns per half-tile

    HALF = 2 * WH * C  # expanded half-row length (8192 f32 = 32 KB)

    # walrus codegen allows exactly ONE sync-wait command per engine
    # instruction (multi-wait is only legal on Drain/EventSemaphore).
    # Tile emits a wait only when the issuing engine has not already
    # observed that semaphore tick through an earlier *real* instruction's
    # wait (NoOps don't count). So every instruction below is budgeted to
    # observe at most one fresh tick, using tiny "absorber" instructions
    # (1-element memsets on DVE, 2-element probe copies on ACT, 4-byte
    # writes on SP) to pre-observe everything else.
    #
    # Loads issue from the SP HWDGE ring and stores from the ACT ring so
    # load prefetch is never blocked behind a store's data wait; each
    # store half fires as soon as its own DVE copy finishes.
    with TileContext(nc) as tc:
        with (
            tc.tile_pool(name="pin", bufs=2) as pin,
            tc.tile_pool(name="pout", bufs=2) as pout,
            tc.tile_pool(name="pdummy", bufs=1) as pdummy,
        ):
            dummy = pdummy.tile([H, 4 * N_ITERS], F32, name="dummy")
            vdummy = pdummy.tile([H, 3 * N_ITERS], F32, name="vdummy")
            pooldummy = pdummy.tile([1, N_ITERS], F32, name="pooldummy")
            spdummy = pdummy.tile([1, 16], F32, name="spdummy")
            last_pabs = None
            st_los = []
            st_his = []
            aabs1s = []
            dmas = []
            cps_all = []
            k = 0
            for t, x in enumerate((xr, xi)):
                for b in range(BPC):
                    # partition i holds input row i, feeding output rows 2i, 2i+1
                    ov = out[t, b].rearrange("(i r) w c -> i r (w c)", r=2)
                    for h in range(2):
                        tin = pin.tile([H, WH * C], F32, name="tin")
                        # Pool-side absorber (gpsimd memset = a real engine
                        # instruction): observe DVE at the newest finished
                        # copy so the load's WAR on its recycled tin slot
                        # (and the slot-release bundle, which lands later
                        # on the DVE timeline than the slot's accessors)
                        # needs no fresh DVE wait.
                        if k >= 2:
                            pabs = nc.gpsimd.memset(pooldummy[:1, k : k + 1], 0.0)
                            add_dep_helper(
                                pabs.ins, cps_all[-1].ins, sync=True,
                                reason="Pool observes DVE for load WAR",
                            )
                            last_pabs = pabs
                        else:
                            pabs = None
                        ld = nc.gpsimd.dma_start(
                            out=tin, in_=x[b, :, h * WH : (h + 1) * WH, :]
                        )
                        if pabs is not None:
                            add_dep_helper(
                                ld.ins, pabs.ins, sync=False,
                                reason="absorber runs before load",
                            )
                        tout = pout.tile([H, 2 * HALF], F32, name="tout")
                        # DVE-side absorbers: per-iter distinct scratch
                        # cells (no WAW chains), not touching tout (the
                        # slot-release bundle must land on cp0, after the
                        # absorbers already observed all of it).
                        vabs1 = nc.vector.memset(vdummy[:1, 3 * k : 3 * k + 1], 0.0)
                        vabs2 = nc.vector.memset(
                            vdummy[:1, 3 * k + 1 : 3 * k + 2], 0.0
                        )
                        vabs3 = nc.vector.memset(
                            vdummy[:1, 3 * k + 2 : 3 * k + 3], 0.0
                        )
                        if k >= 2:
                            add_dep_helper(
                                vabs1.ins, st_los[k - 2].ins, sync=True,
                                reason="absorb tout slot WAR (store-lo lane)",
                            )
                            add_dep_helper(
                                vabs2.ins, st_his[k - 2].ins, sync=True,
                                reason="absorb tout slot WAR (store-hi lane)",
                            )
                        if k >= 1:
                            add_dep_helper(
                                vabs3.ins, aabs1s[k - 1].ins, sync=True,
                                reason="absorb probe WAR (ACT sem)",
                            )
                        src = (
                            tin.rearrange("p (w c) -> p w c", c=C)
                            .unsqueeze(2)
                            .broadcast_to([H, WH, 2, C])
                        )
                        cps = []
                        for r in range(2):
                            dst = tout[:, r * HALF : (r + 1) * HALF].rearrange(
                                "p (w s c) -> p w s c", s=2, c=C
                            )
                            cp = nc.vector.tensor_copy(out=dst, in_=src)
                            for vb in (vabs1, vabs2, vabs3):
                                add_dep_helper(
                                    cp.ins, vb.ins, sync=False,
                                    reason="absorbers run before copies",
                                )
                            cps.append(cp)
                        add_dep_helper(
                            cps[1].ins, cps[0].ins, sync=True,
                            reason="DVE self-sem watermark",
                        )
                        # Each store half fires right after its own copy;
                        # a 2-element ACT probe of that copy's region
                        # absorbs the DVE data wait first.
                        aabs0 = nc.scalar.copy(
                            out=dummy[:1, 4 * k : 4 * k + 2],
                            in_=tout[:1, 0:2],
                        )
                        st_lo = nc.scalar.dma_start(
                            out=ov[:, 0, h * HALF : (h + 1) * HALF],
                            in_=tout[:, :HALF],
                        )
                        add_dep_helper(
                            st_lo.ins, aabs0.ins, sync=False,
                            reason="probe runs before store",
                        )
                        aabs1 = nc.scalar.copy(
                            out=dummy[:1, 4 * k + 2 : 4 * k + 4],
                            in_=tout[:1, HALF : HALF + 2],
                        )
                        st_hi = nc.scalar.dma_start(
                            out=ov[:, 1, h * HALF : (h + 1) * HALF],
                            in_=tout[:, HALF:],
                        )
                        add_dep_helper(
                            st_hi.ins, aabs1.ins, sync=False,
                            reason="probe runs before store",
                        )
                        st_los.append(st_lo)
                        st_his.append(st_hi)
                        aabs1s.append(aabs1)
                        dmas.extend([ld, st_lo, st_hi])
                        cps_all.extend(cps)
                        k += 1
            # Kernel-tail absorbers: Tile's final SP drain waits on every
            # outstanding proc (DVE + ACT + 8 DMA lanes = 10 waits), but a
            # multi-wait drain lowers to a 1-wait NOP struct when cheap.
            # Pre-observe each proc with one 4-byte SP write per tick.
            tail_deps = dmas[-8:] + [aabs1s[-1], cps_all[-1], last_pabs]
            for j, dep in enumerate(tail_deps):
                wr = nc.sync.write(spdummy[:1, j : j + 1], b"\x00\x00\x00\x00")
                add_dep_helper(
                    wr.ins, dep.ins, sync=True,
                    reason="pre-observe outstanding procs for tail drain",
                )
    return nc


_NC_CACHE: bass.Bass | None = None


def _get_nc() -> bass.Bass:
    global _NC_CACHE
    if _NC_CACHE is None:
        _NC_CACHE = _build()
    return _NC_CACHE


def _run(x_real: np.ndarray, x_imag: np.ndarray, **spmd_kwargs):
    x_real = np.ascontiguousarray(np.asarray(x_real, dtype=np.float32))
    x_imag = np.ascontiguousarray(np.asarray(x_imag, dtype=np.float32))
    assert x_real.shape == (B, H, W, C), x_real.shape
    assert x_imag.shape == (B, H, W, C), x_imag.shape
    in_maps = [
        {
            "x_real": x_real[c * BPC : (c + 1) * BPC],
            "x_imag": x_imag[c * BPC : (c + 1) * BPC],
        }
        for c in range(N_CORES)
    ]
    res = run_bass_kernel_spmd(
        _get_nc(), in_maps, core_ids=list(range(N_CORES)), **spmd_kwargs
    )
    full = np.concatenate([r["out"] for r in res.results], axis=1)
    return full, res


def kernel(x_real: np.ndarray, x_imag: np.ndarray) -> np.ndarray:
    full, _ = _run(x_real, x_imag)
    return full
